# revision 58
# baseline (speedup 1.0000x reference)
"""Trainium2 Bass kernel for nn_NodeNet: GNN message passing + 12-qubit TTN.

Collective-free sharding: the host owns the edge partition, so core k
receives exactly the edges whose TARGET node lands in its 128-node
slice - once for mi (targets = idx_i) and once for mo (targets =
idx_o). Both contractions are then fully local and each core computes
only its own 128 nodes:

  stage 1 (gather): beo[j] = X[src_j] via matmul against the one-hot
      source matrix, chunked 128x128 (fp8, exact for 0/1) with moving
      X hi/lo bf16 pairs interleaved per column. Edges are sorted by
      source chunk; runs padded to RPAD=256 (max real run ~170).
      Pairwise tensor_reduce sums hi+lo, one tensor_tensor applies e.
  stage 2: re-split e*X to interleaved bf16 hi/lo (copy + STT + copy).
  stage 3 (scatter): mi[n] = sum_j beo[j] * RiT[j, n] with stationary
      one-hot fp8 blocks and moving beo hi/lo [128, 8]; PSUM
      accumulates over the 16 j-blocks, pairwise reduce evicts - the
      output is node-partitioned, no transposes, no collective.

The circuit contracts exactly to per-node Bloch chains (a CNOT maps
the target to (x, az*y, az*z) where az is the control's z-component;
the measurement is <Z_9>). Every linear term collapses to
A*sin(m + phi) with host-precomputed amplitude/phase tables, so the
whole chain is ~30 vector ops on a 36-column sin tile: build scaled
angle columns (q/r pairs plus per-D az duplicates), range-reduce via
f32->i32 RNE round-trip, one Sin activation (scale=2pi), then
D = ampq*q + ampr*r*az, a 4-wide F combine, and a short z9 tail.

Hardware notes baked into the structure:
- DMA completion semaphores tick once per 8-partition batch (~290ns,
  serialized per queue): one big DMA per queue, xs ridden inside the
  roa tensor as raw bytes, and the 512B output transposed to a single
  partition so its DMA completes in 1 batch instead of 16.
- PSUM: one wide accumulation tile per relation (skip_group_check),
  evicted in single wide ops; only one PSUM operand per vector op.
- Custom DVE ops (add_range_wrap etc.) fail codegen in this walrus
  build ("ISA wrong length"); the flag-gated fallbacks are used.
"""

import ml_dtypes
import numpy as np

import bass_rust
import concourse.bass as bass
import concourse.mybir as mybir
import concourse.tile as tile
from concourse.bass_utils import run_bass_kernel_spmd

F32 = mybir.dt.float32
BF16 = mybir.dt.bfloat16
F8 = mybir.dt.float8e4
F32R = mybir.dt.float32r
N_CORES = 8
N, E, D = 1024, 8192, 4
P = 128                  # partitions / nodes per core
NCH = N // P             # 8 node chunks
RPAD = 256               # padded edges per source chunk (max real ~170)
EPAD = NCH * RPAD        # 2048 padded edges per core per relation
NB = EPAD // P           # 16 j-blocks
PI = float(np.pi)

_SCOPES = False          # test harness can flip on for phase attribution
_FUSED_WRAP = False  # custom-DVE InstISA broken in this walrus build
_FUSED_AFFINE = False       # use custom-DVE fused ops (add_range_wrap etc.)

_BLOCKS = [(0, 1, (0, 1)), (2, 3, (3, 2)), (4, 5, (4, 5)), (6, 7, (7, 6)),
           (8, 9, (8, 9)), (10, 11, (11, 10)), (1, 2, (1, 2)), (5, 6, (6, 5)),
           (9, 10, (10, 9)), (2, 5, (2, 5)), (5, 9, (5, 9))]

# A-layer blocks 0..5: (target rot idx, ctrl rot idx, target wire, ctrl wire)
A_INFO = []
for _b, (_w1, _w2, (_c, _t)) in enumerate(_BLOCKS[:6]):
    A_INFO.append((2 * _b if _t == _w1 else 2 * _b + 1,
                   2 * _b if _c == _w1 else 2 * _b + 1, _t, _c))

# sin30 layout: 6 groups of [q,r]-pairs over A-blocks [0,3,5,1,2,4]
# (b3 and b2 carry 2 and 4 D's), then 6 ctrl cols.
# D12 = [az6, az7, az7, az8, s9a, s9b, p0a, p0b, p1a, p1b, ua, ub]
_GROUPS = [(0, 2, 0), (2, 4, 3), (6, 2, 5), (8, 4, 1), (12, 8, 2),
           (20, 4, 4)]          # (m30 start, width, A-block)

# smalls column layout
SM_X = 0                 # own-node X angle cols (4)
SM_EA = 4                # e for rel A, replicated x4: [128, 64]
SM_EB = 68
SM_PHI = 132             # phi36: q/r pairs 0:24, az dups 24:36
SM_AQ = 168              # q amplitudes per D (12)
SM_AR = 180              # r amplitudes per D (12, ctrl amp folded)
SM_W = 192

# ---------------------------------------------------------------------------
# Host-side circuit-constant preparation
# ---------------------------------------------------------------------------

_PAULI = np.array([
    [[0, 1], [1, 0]],
    [[0, -1j], [1j, 0]],
    [[1, 0], [0, -1]],
], dtype=np.complex128)


def _rot_so3(p):
    """SO(3) Bloch rotation of Rot(phi, theta, omega) = RZ(om) RY(th) RZ(phi)."""
    phi, th, om = float(p[0]), float(p[1]), float(p[2])
    c, s = np.cos(th / 2), np.sin(th / 2)
    U = np.array([
        [np.exp(-0.5j * (phi + om)) * c, -np.exp(0.5j * (phi - om)) * s],
        [np.exp(-0.5j * (phi - om)) * s, np.exp(0.5j * (phi + om)) * c],
    ])
    R = np.empty((3, 3))
    for i in range(3):
        for j in range(3):
            R[i, j] = 0.5 * np.real(
                np.trace(_PAULI[i] @ U @ _PAULI[j] @ U.conj().T))
    return R


def _pack_tables(theta):
    """phi30/amp30 for the amplitude-phase sin tile (see module docstring)."""
    th = np.asarray(theta, np.float64)
    R = [_rot_so3(th[3 * k:3 * k + 3]) for k in range(23)]

    def split_ab(row2, Rt):
        return row2[0] * Rt[0, :], row2[1] * Rt[1, :] + row2[2] * Rt[2, :]

    a_s9, b_s9 = split_ab(R[18][2], R[13])
    v0 = R[20][2, 0] * R[19][0, :]
    v1 = R[20][2, 1] * R[19][1, :] + R[20][2, 2] * R[19][2, :]
    a_p0, b_p0 = split_ab(v0, R[14])
    a_p1, b_p1 = split_ab(v1, R[14])
    a_u, b_u = split_ab(R[21][2], R[16])

    D_order = [(R[12][2], 0), (R[15][2], 3), (R[15][2], 3), (R[17][2], 5),
               (a_s9, 1), (b_s9, 1), (a_p0, 2), (b_p0, 2), (a_p1, 2),
               (b_p1, 2), (a_u, 4), (b_u, 4)]

    phi36 = np.zeros(36)
    ampq = np.zeros(12)
    ampr = np.zeros(12)
    for j, (kappa, b) in enumerate(D_order):
        Rt, Rc = R[A_INFO[b][0]], R[A_INFO[b][1]]
        cs, cc = kappa[0] * Rt[0, 0], kappa[0] * Rt[0, 2]
        ampq[j] = np.hypot(cs, cc)
        phi36[2 * j] = np.arctan2(cc, cs)
        cs = kappa[1] * Rt[1, 0] + kappa[2] * Rt[2, 0]
        cc = kappa[1] * Rt[1, 2] + kappa[2] * Rt[2, 2]
        ampr[j] = np.hypot(cs, cc) * np.hypot(Rc[2, 0], Rc[2, 2])
        phi36[2 * j + 1] = np.arctan2(cc, cs)
        # az duplicate col for this D (ctrl of its A-block)
        phi36[24 + j] = np.arctan2(Rc[2, 2], Rc[2, 0])
    return (phi36.astype(np.float32), ampq.astype(np.float32),
            ampr.astype(np.float32))


# ---------------------------------------------------------------------------
# Walrus workaround: this build rejects >1 sync-wait per instruction
# ---------------------------------------------------------------------------


def _split_multi_waits(nc):
    for f in nc.m.functions:
        for bb in f.blocks:
            out = []
            for inst in bb.instructions:
                si = inst.sync_info
                if si is not None and si.on_wait and len(si.on_wait) > 1:
                    waits = list(si.on_wait)
                    for i, w in enumerate(waits[:-1]):
                        out.append(mybir.InstNoOp(
                            name=f"{inst.name}_wsplit{i}",
                            engine=inst.engine,
                            ins=[], outs=[],
                            sync_info=bass_rust.SyncInfo(
                                on_wait=[w], on_update=[]),
                        ))
                    inst.sync_info = bass_rust.SyncInfo(
                        on_wait=[waits[-1]], on_update=list(si.on_update))
                out.append(inst)
            bb.instructions = out


# ---------------------------------------------------------------------------
# Device kernel
# ---------------------------------------------------------------------------


def _build_nc():
    nc = bass.Bass("TRN2", target_bir_lowering=False, num_devices=N_CORES)

    roa_d = nc.declare_dram_parameter("roa", [P, 2 * EPAD + 128], F8,
                                      isOutput=False)
    rit_d = nc.declare_dram_parameter("rit", [P, 2 * EPAD], F8,
                                      isOutput=False)
    sm_d = nc.declare_dram_parameter("smalls", [P, SM_W], F32, isOutput=False)
    out_d = nc.declare_dram_parameter("out", [1, P], F32, isOutput=True)

    MUL = mybir.AluOpType.mult
    ADD = mybir.AluOpType.add

    with tile.TileContext(nc) as tc:
        import contextlib
        scope = (tc.spectator_scope if _SCOPES else
                 (lambda name: contextlib.nullcontext()))
        with (
            tc.tile_pool(name="sb", bufs=1) as sb,
            tc.tile_pool(name="s1p", bufs=1, space="PSUM") as s1p,
            tc.tile_pool(name="accp", bufs=1, space="PSUM") as accp,
        ):
            sc = scope("ld")
            sc.__enter__()
            roa_t = sb.tile([P, 2 * EPAD + 128], F8, name="roa")
            rit_t = sb.tile([P, 2 * EPAD], F8, name="rit")
            roa_sb = {"a": roa_t[:, 0:EPAD], "b": roa_t[:, EPAD:]}
            rit_sb = {"a": rit_t[:, 0:EPAD], "b": rit_t[:, EPAD:]}
            xs_sb = roa_t[:, 2 * EPAD:].bitcast(BF16)
            sm_sb = sb.tile([P, SM_W], F32, name="sm_sb")
            # one big DMA per queue: completion semaphores pace ~290ns per
            # 8-partition batch and serialize per queue
            nc.sync.dma_start(roa_t[:], roa_d[:])
            nc.scalar.dma_start(sm_sb[:], sm_d[:])
            nc.gpsimd.dma_start(rit_t[:], rit_d[:])

            # preload the ACT Sin table while DMAs stream; warm the PE
            warm = sb.tile([P, 8], F32, name="warm")
            nc.vector.memset(warm[:], 0.0)
            nc.scalar.activation(warm[:, 0:1], warm[:, 0:1],
                                 mybir.ActivationFunctionType.Sin)
            from concourse.masks import make_identity
            ident = sb.tile([P, P], F32, name="ident")
            make_identity(nc, ident)
            warm16 = sb.tile([P, P], BF16, name="warm16")
            nc.vector.memset(warm16[:], 0.0)
            for i in range(3):
                wp = s1p.tile([P, 8], F32, name=f"warm_ps{i}", tag="s1")
                nc.tensor.matmul(wp[:], warm16[:], warm16[:, 0:8],
                                 start=True, stop=True)
            sc.__exit__(None, None, None)

            # ---- stage 1 + 2 per relation --------------------------------
            sc = scope("s1")
            sc.__enter__()
            ENG = [nc.vector, nc.gpsimd]
            bhl = {}
            for ri, r in enumerate("ab"):
                beo = sb.tile([P, NB * D], F32, name=f"beo_{r}")
                eoff = SM_EA if ri == 0 else SM_EB
                ps = s1p.tile([P, NB * 8], F32, name=f"s1ps_{r}",
                              tag=f"s1{r}")
                for m in range(NB):
                    nc.tensor.matmul(
                        ps[:, m * 8:(m + 1) * 8],
                        roa_t[:, (0 if r == 'a' else EPAD) + m * P:(0 if r == 'a' else EPAD) + (m + 1) * P],
                        xs_sb[:, (m // 2) * 8:(m // 2) * 8 + 8],
                        start=True, stop=True, skip_group_check=True)
                # xs cols interleave [hi, lo] per d: pair-average, then a
                # single scale by 2*e (the x2 is folded into e host-side)
                pooled = sb.tile([P, NB * D], F32, name=f"pool_{r}")
                nc.vector.tensor_reduce(
                    pooled.rearrange("p (m d) -> p m d", d=D),
                    ps.rearrange("p (md two) -> p md two", two=2),
                    mybir.AxisListType.X, ADD)
                nc.vector.tensor_tensor(
                    beo[:], pooled[:], sm_sb[:, eoff:eoff + NB * D], MUL)
                # split into bf16 hi/lo pairs, interleaved per d
                hl = sb.tile([P, NB * 8], BF16, name=f"bhl_{r}")
                hl3 = hl.rearrange("p (m c) -> p m c", c=8)
                hi_view, lo_view = hl3[:, :, 0:8:2], hl3[:, :, 1:8:2]
                beo3 = beo.rearrange("p (m d) -> p m d", d=D)
                brs = sb.tile([P, NB * D], F32, name=f"brs_{r}")
                brs3 = brs.rearrange("p (m d) -> p m d", d=D)
                nc.gpsimd.tensor_copy(hi_view, beo3)
                nc.vector.scalar_tensor_tensor(
                    brs3, hi_view, -1.0, beo3, MUL, ADD)
                nc.gpsimd.tensor_copy(lo_view, brs3)
                bhl[r] = hl
            sc.__exit__(None, None, None)

            # ---- stage 3: scatter to own nodes ---------------------------
            sc = scope("s3")
            sc.__enter__()
            # mm holds (mi|mo)/2 via pair-averaging; the build step's
            # uniform 1/pi scale (with phi and X pre-halved on the host)
            # absorbs the missing factor 2
            mm = sb.tile([P, 8], F32, name="mm")
            for ri, r in enumerate("ab"):
                acc = accp.tile([P, 8], F32, name=f"acc_{r}", tag=f"acc{r}")
                for m in range(NB):
                    nc.tensor.matmul(
                        acc[:], rit_t[:, (0 if r == 'a' else EPAD) + m * P:(0 if r == 'a' else EPAD) + (m + 1) * P],
                        bhl[r][:, m * 8:(m + 1) * 8],
                        start=(m == 0), stop=(m == NB - 1))
                nc.vector.tensor_reduce(
                    mm[:, ri * 4:(ri + 1) * 4].rearrange(
                        "p (d one) -> p d one", one=1),
                    acc.rearrange("p (d two) -> p d two", d=D, two=2),
                    mybir.AxisListType.X, ADD)
            sc.__exit__(None, None, None)

            # ---- circuit ------------------------------------------------
            sc = scope("ci")
            sc.__enter__()

            def phi(s, w):
                return sm_sb[:, SM_PHI + s:SM_PHI + s + w]

            # m36 build: q/r pairs 0:24 (per-group target angle), az dups
            # 24:36 (per-D ctrl angle). Vector writes (phi+m)/2pi fused;
            # scalar writes phi+m and a whole-tile rescale follows.
            m36 = sb.tile([P, 36], F32, name="m36")
            RPIX = float(1.0 / (2 * PI))
            # target angle source per group: wires [1, 6, 10, 2, 5, 9]
            tsrc = [mm[:, 1:2], mm[:, 6:7], sm_sb[:, SM_X + 2:SM_X + 3],
                    mm[:, 2:3], mm[:, 5:6], sm_sb[:, SM_X + 1:SM_X + 2]]
            # az-dup runs: (start, width, source) ctrl wires [0,7,11,3,4,8]
            azrun = [(24, 1, mm[:, 0:1]), (25, 2, mm[:, 7:8]),
                     (27, 1, sm_sb[:, SM_X + 3:SM_X + 4]),
                     (28, 2, mm[:, 3:4]), (30, 4, mm[:, 4:5]),
                     (34, 2, sm_sb[:, SM_X:SM_X + 1])]
            vec_jobs = []
            for g, (s, w, _b) in enumerate(_GROUPS):
                vec_jobs.append((s, w, tsrc[g]))
            sc_jobs = list(azrun)
            for s, w, srcap in vec_jobs:
                nc.vector.tensor_scalar(m36[:, s:s + w], phi(s, w), srcap,
                                        RPIX, ADD, mybir.AluOpType.mult)
            for s, w, srcap in sc_jobs:
                nc.scalar.add(m36[:, s:s + w], phi(s, w), srcap)
            nc.gpsimd.tensor_scalar(
                m36[:, 24:36], m36[:, 24:36], RPIX, None, MUL)

            # wrap: m' - rne(m') in [-0.5, 0.5], Sin applies the 2pi scale
            t_i = sb.tile([P, 36], mybir.dt.int32, name="t_i")
            t_r = sb.tile([P, 36], F32, name="t_r")
            nc.vector.tensor_copy(t_i[:], m36[:])
            nc.vector.tensor_copy(t_r[:], t_i[:])
            nc.vector.scalar_tensor_tensor(
                m36[:], t_r[:], -1.0, m36[:], MUL, ADD)
            s36 = sb.tile([P, 36], F32, name="s36")
            nc.scalar.activation(s36[:], m36[:],
                                 mybir.ActivationFunctionType.Sin,
                                 scale=float(2 * PI))

            # D12 = ampq*q + ampr*r*az  (q/r strided, az contiguous)
            d12 = sb.tile([P, 12], F32, name="d12")
            t12 = sb.tile([P, 12], F32, name="t12")
            nc.vector.tensor_tensor(t12[:], s36[:, 1:24:2], s36[:, 24:36],
                                    MUL)
            nc.vector.tensor_tensor(t12[:], t12[:],
                                    sm_sb[:, SM_AR:SM_AR + 12], MUL)
            nc.vector.tensor_tensor(d12[:], s36[:, 0:24:2],
                                    sm_sb[:, SM_AQ:SM_AQ + 12], MUL)
            nc.vector.tensor_tensor(d12[:], d12[:], t12[:], ADD)

            # F = [s9, p0, p1, u]; z9 = (p0 + s9*p1)*u
            f4 = sb.tile([P, 8], F32, name="f4")
            nc.vector.tensor_tensor(f4[:, 4:8], d12[:, 5:12:2], d12[:, 0:4],
                                    MUL)
            nc.vector.tensor_tensor(f4[:, 0:4], d12[:, 4:12:2], f4[:, 4:8],
                                    ADD)
            t2 = sb.tile([P, 2], F32, name="t2")
            nc.vector.tensor_tensor(t2[:, 0:1], f4[:, 0:1], f4[:, 2:3], MUL)
            nc.vector.tensor_tensor(t2[:, 1:2], f4[:, 1:2], t2[:, 0:1], ADD)
            res = sb.tile([P, 1], F32, name="res")
            nc.vector.tensor_tensor(res[:], t2[:, 1:2], f4[:, 3:4], MUL)
            nc.vector.tensor_scalar(res[:], res[:], -PI, PI, MUL, ADD)
            # single-partition out: 1 semaphore batch instead of 16
            rT_ps = accp.tile([1, P], F32, name="rT_ps", tag="rT")
            nc.tensor.transpose(rT_ps[:], res[:], ident[:])
            rT = sb.tile([1, P], F32, name="rT")
            nc.vector.tensor_copy(rT[:], rT_ps[:])
            nc.sync.dma_start(out_d[:], rT[:])
            sc.__exit__(None, None, None)

    return nc


_NC_CACHE = {}
_RUN_KWARGS = {}      # test harness can set e.g. {"trace": True}
_LAST_RESULTS = []    # BassKernelResults of the most recent run


def _get_nc():
    if "nc" not in _NC_CACHE:
        nc = _build_nc()
        _split_multi_waits(nc)
        _NC_CACHE["nc"] = nc
    return _NC_CACHE["nc"]


def _shard_rel(idx_t, idx_s, e):
    """Per-core (roa, rita, e64) for one relation.

    Core k owns edges with idx_t in its node slice, sorted by source
    chunk; runs padded to RPAD. roa[src%128, j] = 1 (stage-1 stationary,
    partition = node-in-chunk), rita[j%128, (j//128)*128 + tgt%128] = 1
    (stage-3 stationary, partition = j-in-block, host p-major layout).
    """
    f8 = ml_dtypes.float8_e4m3fn
    outs = []
    for k in range(N_CORES):
        ed = np.where(idx_t // P == k)[0]
        sc = idx_s[ed] // P
        order = np.argsort(sc, kind="stable")
        ed, sc = ed[order], sc[order]
        counts = np.bincount(sc, minlength=NCH)
        if counts.max() > RPAD:
            raise ValueError(f"source-chunk run {counts.max()} > RPAD")
        starts = np.searchsorted(sc, np.arange(NCH))
        j = np.arange(len(ed)) - starts[sc] + sc * RPAD
        roa = np.zeros((P, EPAD), np.float32)
        roa[idx_s[ed] % P, j] = 1.0
        rita = np.zeros((P, EPAD), np.float32)
        rita[j % P, (j // P) * P + idx_t[ed] % P] = 1.0
        e16 = np.zeros((P, NB), np.float32)
        e16[j % P, j // P] = e[ed]
        outs.append((roa.astype(f8), rita.astype(f8),
                     np.repeat(e16, D, axis=1)))
    return outs


def kernel(X, e, Ri, Ro, theta):
    X = np.ascontiguousarray(np.asarray(X, np.float32))
    e = np.ascontiguousarray(np.asarray(e, np.float32))
    Ri = np.asarray(Ri, np.float32)
    Ro = np.asarray(Ro, np.float32)
    theta = np.asarray(theta, np.float32)

    bf = ml_dtypes.bfloat16
    idx_i = np.argmax(Ri, axis=0)
    idx_o = np.argmax(Ro, axis=0)

    # rel A feeds mi (targets idx_i, sources idx_o); rel B feeds mo
    sh_a = _shard_rel(idx_i, idx_o, e)
    sh_b = _shard_rel(idx_o, idx_i, e)

    xh = X.astype(bf).astype(np.float32)
    xl = X - xh
    xs = np.zeros((P, NCH, 8), np.float32)
    xs[:, :, 0:8:2] = xh.reshape(NCH, P, D).transpose(1, 0, 2)
    xs[:, :, 1:8:2] = xl.reshape(NCH, P, D).transpose(1, 0, 2)
    xs = np.ascontiguousarray(xs.reshape(P, NCH * 8).astype(bf))

    phi36, ampq, ampr = _pack_tables(theta)

    in_maps = []
    for k in range(N_CORES):
        sm = np.zeros((P, SM_W), np.float32)
        sm[:, SM_X:SM_X + 4] = X[k * P:(k + 1) * P]
        sm[:, SM_EA:SM_EA + NB * D] = sh_a[k][2]
        sm[:, SM_EB:SM_EB + NB * D] = sh_b[k][2]
        sm[:, SM_PHI:SM_PHI + 36] = phi36[None, :]
        sm[:, SM_AQ:SM_AQ + 12] = ampq[None, :]
        sm[:, SM_AR:SM_AR + 12] = ampr[None, :]
        xs_bytes = xs.view(np.uint8).reshape(P, 128).view(
            ml_dtypes.float8_e4m3fn)
        in_maps.append({
            "roa": np.ascontiguousarray(np.concatenate(
                [sh_a[k][0], sh_b[k][0], xs_bytes], axis=1)),
            "rit": np.ascontiguousarray(
                np.concatenate([sh_a[k][1], sh_b[k][1]], axis=1)),
            "smalls": np.ascontiguousarray(sm),
        })

    nc = _get_nc()
    res = run_bass_kernel_spmd(nc, in_maps, core_ids=list(range(N_CORES)),
                               **_RUN_KWARGS)
    _LAST_RESULTS.clear()
    _LAST_RESULTS.append(res)
    return np.concatenate(
        [res.results[k]["out"].reshape(-1) for k in range(N_CORES)]
    ).astype(np.float32)
